# revision 8
# baseline (speedup 1.0000x reference)
"""Self-contained Trainium2 Bass kernel for a 12-head attention layer.

Problem: x[4,2048,768] -> attention(QKV projections, softmax, context),
NUM_HEADS=12, SIZE_PER_HEAD=64, additive mask from mask[4,2048].

Sharding over 8 NeuronCores: core c handles batch b=c//2 and head-group
hg=c%2 (6 heads, 384 feature columns).  Everything is local per core:
no collectives.  Host-side prep per core:
  - xT_aug [769,2048] bf16 = concat(x[b].T, ones-row)   (bias via matmul)
  - wq [769,384] bf16 = concat(Wq[:,cols]/8, bq[cols]/8)  (1/sqrt(64) folded)
  - wk [769,384] bf16 = concat(Wk[:,cols], bk[cols])
  - wv [769,390] bf16: head-major with a 65th "ones" column per head so the
    context matmul also produces the softmax denominator (row 64 of ctx').
  - adder [128,16] f32 = (mask[b]-1)*10000 laid out per T-tile (ACT bias).

On-chip per core (all matmuls bf16, PSUM f32):
  QT,KT [384,2048] = W.T @ xT_aug      (feature-major projections)
  V'    [2048,390] = xT_aug.T @ wv     (token-major, with ones cols)
  per head h, F-block of 1024, T-tile of 128:
    S^T[T,F] psum  = KT_h^T @ QT_h     (K=64 contraction)
    exp sbuf bf16  = ACT Exp(S^T + adder[T])   (mask as per-partition bias)
    ctx'[65,F] psum += V'_h[T-tile].T @ exp    (K=128; row 64 = denominator)
  normalize: reciprocal of denominators (DVE, [128,96] layout via sbuf-sbuf
  DMA gather/scatter), broadcast via K=1 matmul, fused multiply, DMA out.

Output per core: ctx^T [384,2048] f32; host transposes/concats to [4,2048,768].
"""

import numpy as np
import ml_dtypes

B, S, D = 4, 2048, 768
H, DH = 12, 64
HL = 6          # heads per core
DL = HL * DH    # 384 feature columns per core
NCORES = 8
P = 128
KO = 6          # full k-subtiles of the 768 contraction
NT = S // P     # 16 T-tiles
FB = 1024       # F block size
NFB = S // FB   # 2 F blocks
NSUB = FB // 512  # 512-wide matmul chunks per F block

_CACHE = {}


def _build(with_bias=False):
    import concourse.mybir as mybir
    import concourse.tile as tile
    from concourse import bacc

    dt = mybir.dt
    Exp = mybir.ActivationFunctionType.Exp
    Alu = mybir.AluOpType

    nc = bacc.Bacc("TRN2", target_bir_lowering=False, debug=False,
                   num_devices=NCORES)

    xT = nc.dram_tensor("xT", [D + 1, S], dt.bfloat16, kind="ExternalInput")
    wq = nc.dram_tensor("wq", [D + 1, DL], dt.bfloat16, kind="ExternalInput")
    wk = nc.dram_tensor("wk", [D + 1, DL], dt.bfloat16, kind="ExternalInput")
    wv = nc.dram_tensor("wv", [D + 1, HL * (DH + 1)], dt.bfloat16,
                        kind="ExternalInput")
    adder = nc.dram_tensor("adder", [P, NT], dt.float32, kind="ExternalInput")
    out = nc.dram_tensor("out", [DL, S], dt.float32, kind="ExternalOutput")

    with tile.TileContext(nc) as tc:
        with (
            tc.tile_pool(name="persist", bufs=1) as sb,
            tc.tile_pool(name="work", bufs=4) as work,
            tc.tile_pool(name="fin", bufs=3) as fin,
            tc.tile_pool(name="ps_s", bufs=2, space="PSUM") as ps_s,
            tc.tile_pool(name="ps_c", bufs=2, space="PSUM") as ps_c,
        ):
            # ---- input DMA ----
            xTs = sb.tile([P, KO + 1, S], dt.bfloat16, tag="xTs")
            for ko in range(KO):
                nc.sync.dma_start(
                    xTs[:, ko, :], xT.ap()[ko * P:(ko + 1) * P, :])
            nc.sync.dma_start(xTs[0:1, KO, :], xT.ap()[D:D + 1, :])

            wqs = sb.tile([P, KO + 1, DL], dt.bfloat16, tag="wqs")
            wks = sb.tile([P, KO + 1, DL], dt.bfloat16, tag="wks")
            wvs = sb.tile([P, KO + 1, HL * (DH + 1)], dt.bfloat16, tag="wvs")
            for w_dram, w_sb in ((wq, wqs), (wk, wks), (wv, wvs)):
                cols = w_dram.shape[1]
                nc.sync.dma_start(
                    w_sb[:, 0:KO, :],
                    w_dram.ap()[0:D, :].rearrange("(ko p) m -> p ko m", p=P))
                nc.sync.dma_start(w_sb[0:1, KO, :], w_dram.ap()[D:D + 1, :])

            adder_sb = sb.tile([P, NT], dt.float32, tag="adder")
            nc.sync.dma_start(adder_sb[:], adder.ap())

            # persistent projection outputs
            qt = sb.tile([P, 3, S], dt.bfloat16, tag="qt")   # Q^T/8 + bias
            kt = sb.tile([P, 3, S], dt.bfloat16, tag="kt")   # K^T + bias
            vp = sb.tile([P, NT, HL * (DH + 1)], dt.bfloat16, tag="vp")
            # unnormalized ctx' (65 rows per head; row 64 = denominator)
            ctxs = sb.tile([DH + 1, HL, S], dt.float32, tag="ctxs")
            ones64 = sb.tile([1, DH], dt.bfloat16, tag="ones")
            nc.vector.memset(ones64[:], 1.0)

            KE = KO + 1 if with_bias else KO  # k-subtiles incl. optional bias

            # ---- projections: QT/KT m-tiles, V token-major per pair ----
            def proj_qk(w_sb, dst, m):
                for n in range(4):
                    pt = ps_s.tile([P, 512], dt.float32, tag="s", name="pt")
                    for k in range(KE):
                        lhsT = (w_sb[:, k, m * P:(m + 1) * P] if k < KO
                                else w_sb[0:1, k, m * P:(m + 1) * P])
                        rhs = (xTs[:, k, n * 512:(n + 1) * 512] if k < KO
                               else xTs[0:1, k, n * 512:(n + 1) * 512])
                        nc.tensor.matmul(pt[:], lhsT, rhs,
                                         start=(k == 0), stop=(k == KE - 1))
                    nc.vector.tensor_copy(dst[:, m, n * 512:(n + 1) * 512], pt[:])

            def proj_v(mt, hp):
                # always includes the k=768 row: it carries the ones
                # indicator that builds V's 65th (denominator) column.
                w = 2 * (DH + 1)           # one head-pair's V' columns
                c0 = hp * w
                pt = ps_s.tile([P, 512], dt.float32, tag="s", name="pt")
                for k in range(KO + 1):
                    lhsT = (xTs[:, k, mt * P:(mt + 1) * P] if k < KO
                            else xTs[0:1, k, mt * P:(mt + 1) * P])
                    rhs = (wvs[:, k, c0:c0 + w] if k < KO
                           else wvs[0:1, k, c0:c0 + w])
                    nc.tensor.matmul(pt[:, :w], lhsT, rhs,
                                     start=(k == 0), stop=(k == KO))
                nc.vector.tensor_copy(vp[:, mt, c0:c0 + w], pt[:, :w])

            def normalize_pair(hp):
                # denominators of heads (2hp, 2hp+1) -> reciprocal ->
                # K=1 matmul broadcast -> fused multiply -> out
                den = fin.tile([P, 2 * NT], dt.float32, tag="den", bufs=2,
                               name="den")
                for hip in range(2):
                    h = 2 * hp + hip
                    nc.sync.dma_start(den[:, hip * NT:(hip + 1) * NT],
                                      ctxs[DH:DH + 1, h, :])
                rec = fin.tile([P, 2 * NT], dt.float32, tag="rec", bufs=2,
                               name="rec")
                nc.vector.reciprocal(rec[:], den[:])
                recb = fin.tile([P, 2 * NT], dt.bfloat16, tag="recb", bufs=2,
                                name="recb")
                nc.vector.tensor_copy(recb[:], rec[:])
                for hip in range(2):
                    h = 2 * hp + hip
                    rr = fin.tile([1, S], dt.bfloat16, tag="recrow", bufs=2,
                                  name="rr")
                    nc.sync.dma_start(rr[:], recb[:, hip * NT:(hip + 1) * NT])
                    for n in range(S // 512):
                        bc = ps_c.tile([DH, 512], dt.float32, tag="c",
                                       name="bc")
                        nc.tensor.matmul(bc[:], ones64[:],
                                         rr[:, n * 512:(n + 1) * 512],
                                         start=True, stop=True)
                        ot = fin.tile([DH, 512], dt.float32, tag="ot",
                                      bufs=3, name="ot")
                        nc.vector.scalar_tensor_tensor(
                            ot[:], bc[:], 1.0,
                            ctxs[0:DH, h, n * 512:(n + 1) * 512],
                            Alu.bypass, Alu.mult)
                        nc.sync.dma_start(
                            out.ap()[h * DH:(h + 1) * DH,
                                     n * 512:(n + 1) * 512],
                            ot[:])

            def attn_pair(hp):
                # inner loops for head pair (2hp, 2hp+1): scores row-packed
                # (partitions 0-63 / 64-127 run concurrently on PE), exp on
                # ACT with mask bias, ctx' accumulation over T.
                mtile = hp
                for fb in range(NFB):
                    ctx_ps = [
                        ps_c.tile([DH + 1, FB], dt.float32, tag="c",
                                  name="ctx_ps")
                        for _ in range(2)
                    ]
                    exp_tiles = {}

                    def mm_ctx(hip, ti):
                        h = 2 * hp + hip
                        et = exp_tiles.pop((hip, ti))
                        for n in range(NSUB):
                            nc.tensor.matmul(
                                ctx_ps[hip][:, n * 512:(n + 1) * 512],
                                vp[:, ti, h * (DH + 1):(h + 1) * (DH + 1)],
                                et[:, n * 512:(n + 1) * 512],
                                start=(ti == 0), stop=(ti == NT - 1))

                    # per-ti emission [sA, ctxA(prev), expA, sB, ctxB(prev),
                    # expB]: each head's next scores only gate on ITS OWN
                    # previous exp (slot rotation), so ACT runs back-to-back.
                    for ti in range(NT):
                        for hip in range(2):
                            off = hip * DH
                            s_ps = ps_s.tile([P, FB], dt.float32, tag="s",
                                             name="s_ps")
                            for n in range(NSUB):
                                fcol = fb * FB + n * 512
                                nc.tensor.matmul(
                                    s_ps[:, n * 512:(n + 1) * 512],
                                    kt[off:off + DH, mtile,
                                       ti * P:(ti + 1) * P],
                                    qt[off:off + DH, mtile, fcol:fcol + 512],
                                    start=True, stop=True)
                            if ti > 0:
                                mm_ctx(hip, ti - 1)
                            et = work.tile([P, FB], dt.bfloat16, tag="exp",
                                           name="et")
                            nc.scalar.activation(
                                et[:], s_ps[:], Exp,
                                bias=adder_sb[:, ti:ti + 1], scale=1.0)
                            exp_tiles[(hip, ti)] = et
                    mm_ctx(0, NT - 1)
                    mm_ctx(1, NT - 1)

                    # drain ctx' psum to sbuf staging
                    for hip in range(2):
                        h = 2 * hp + hip
                        nc.vector.tensor_copy(
                            ctxs[:, h, fb * FB:(fb + 1) * FB],
                            ctx_ps[hip][:])

            # emission order tuned for overlap: minimal prefix (qt/kt m0 +
            # pair-0 V), then pair-0 attention; later projections and each
            # pair's normalization interleave with the next pair's
            # ACT-bound inner loop.
            proj_qk(wqs, qt, 0)
            proj_qk(wks, kt, 0)
            for mt in range(NT):
                proj_v(mt, 0)
            attn_pair(0)
            proj_qk(wqs, qt, 1)
            proj_qk(wks, kt, 1)
            for mt in range(NT):
                proj_v(mt, 1)
            attn_pair(1)
            normalize_pair(0)
            proj_qk(wqs, qt, 2)
            proj_qk(wks, kt, 2)
            for mt in range(NT):
                proj_v(mt, 2)
            attn_pair(2)
            normalize_pair(1)
            normalize_pair(2)

    nc.compile()
    return nc


def _prep_core_inputs(c, x, Wq, bq, Wk, bk, Wv, bv, mask):
    bf16 = ml_dtypes.bfloat16
    b, hg = c // 2, c % 2
    cols = slice(hg * DL, (hg + 1) * DL)

    xT_aug = np.empty((D + 1, S), dtype=bf16)
    xT_aug[:D] = x[b].T.astype(bf16)
    xT_aug[D] = np.float32(1.0)

    wq_aug = np.empty((D + 1, DL), dtype=bf16)
    wq_aug[:D] = (Wq[:, cols] / 8.0).astype(bf16)
    wq_aug[D] = (bq[cols] / 8.0).astype(bf16)

    wk_aug = np.empty((D + 1, DL), dtype=bf16)
    wk_aug[:D] = Wk[:, cols].astype(bf16)
    wk_aug[D] = bk[cols].astype(bf16)

    wv_aug = np.zeros((D + 1, HL * (DH + 1)), dtype=bf16)
    wv_loc = Wv[:, cols].astype(np.float32)
    bv_loc = bv[cols].astype(np.float32)
    for j in range(HL):
        wv_aug[:D, j * (DH + 1):j * (DH + 1) + DH] = \
            wv_loc[:, j * DH:(j + 1) * DH].astype(bf16)
        wv_aug[D, j * (DH + 1):j * (DH + 1) + DH] = \
            bv_loc[j * DH:(j + 1) * DH].astype(bf16)
        wv_aug[D, j * (DH + 1) + DH] = np.float32(1.0)

    add = ((mask[b].astype(np.float32) - 1.0) * 10000.0)
    adder_t = add.reshape(NT, P).T.copy()   # [128,16]: [p, ti] = add[ti*128+p]

    return {"xT": xT_aug, "wq": wq_aug, "wk": wk_aug, "wv": wv_aug,
            "adder": np.ascontiguousarray(adder_t, dtype=np.float32)}


def kernel(x, Wq, bq, Wk, bk, Wv, bv, mask, _trace=False):
    from concourse.bass_utils import run_bass_kernel_spmd

    x = np.asarray(x, dtype=np.float32)
    Wq = np.asarray(Wq, dtype=np.float32)
    bq = np.asarray(bq, dtype=np.float32)
    Wk = np.asarray(Wk, dtype=np.float32)
    bk = np.asarray(bk, dtype=np.float32)
    Wv = np.asarray(Wv, dtype=np.float32)
    bv = np.asarray(bv, dtype=np.float32)
    mask = np.asarray(mask)

    with_bias = bool(bq.any() or bk.any() or bv.any())
    key = ("nc", with_bias)
    if key not in _CACHE:
        _CACHE[key] = _build(with_bias=with_bias)
    nc = _CACHE[key]

    in_maps = [_prep_core_inputs(c, x, Wq, bq, Wk, bk, Wv, bv, mask)
               for c in range(NCORES)]
    res = run_bass_kernel_spmd(nc, in_maps, core_ids=list(range(NCORES)),
                               trace=_trace)
    if _trace:
        _CACHE["last_result"] = res

    full = np.empty((B, S, D), dtype=np.float32)
    for c in range(NCORES):
        b, hg = c // 2, c % 2
        full[b, :, hg * DL:(hg + 1) * DL] = res.results[c]["out"].T
    return full


# revision 9
# speedup vs baseline: 1.3396x; 1.3396x over previous
"""Self-contained Trainium2 Bass kernel for a 12-head attention layer.

Problem: x[4,2048,768] -> attention(QKV projections, softmax, context),
NUM_HEADS=12, SIZE_PER_HEAD=64, additive mask from mask[4,2048].

Sharding over 8 NeuronCores: core c handles batch b=c//2 and head-group
hg=c%2 (6 heads, 384 feature columns).  Everything is local per core:
no collectives.  Host-side prep per core:
  - xT_aug [769,2048] bf16 = concat(x[b].T, ones-row)   (bias via matmul)
  - wq [769,384] bf16 = concat(Wq[:,cols]/8, bq[cols]/8)  (1/sqrt(64) folded)
  - wk [769,384] bf16 = concat(Wk[:,cols], bk[cols])
  - wv [769,390] bf16: head-major with a 65th "ones" column per head so the
    context matmul also produces the softmax denominator (row 64 of ctx').
  - adder [128,16] f32 = (mask[b]-1)*10000 laid out per T-tile (ACT bias).

On-chip per core (all matmuls bf16, PSUM f32):
  QT,KT [384,2048] = W.T @ xT_aug      (feature-major projections)
  V'    [2048,390] = xT_aug.T @ wv     (token-major, with ones cols)
  per head h, F-block of 1024, T-tile of 128:
    S^T[T,F] psum  = KT_h^T @ QT_h     (K=64 contraction)
    exp sbuf bf16  = ACT Exp(S^T + adder[T])   (mask as per-partition bias)
    ctx'[65,F] psum += V'_h[T-tile].T @ exp    (K=128; row 64 = denominator)
  normalize per segment, fully off the TensorEngine: reciprocal of
  denominators (DVE, [128,*] layout via sbuf-sbuf DMA gather/scatter),
  gpsimd partition_broadcast of the reciprocal row, DVE multiply, DMA out.

Output per core: ctx^T [384,2048] f32; host transposes/concats to [4,2048,768].
"""

import numpy as np
import ml_dtypes

B, S, D = 4, 2048, 768
H, DH = 12, 64
HL = 6          # heads per core
DL = HL * DH    # 384 feature columns per core
NCORES = 8
P = 128
KO = 6          # full k-subtiles of the 768 contraction
NT = S // P     # 16 T-tiles
FB = 1024       # F block size
NFB = S // FB   # 2 F blocks
NSUB = FB // 512  # 512-wide matmul chunks per F block

_CACHE = {}


def _build(with_bias=False):
    import concourse.mybir as mybir
    import concourse.tile as tile
    from concourse import bacc

    dt = mybir.dt
    Exp = mybir.ActivationFunctionType.Exp
    Alu = mybir.AluOpType

    nc = bacc.Bacc("TRN2", target_bir_lowering=False, debug=False,
                   num_devices=NCORES)

    xT = nc.dram_tensor("xT", [D + 1, S], dt.bfloat16, kind="ExternalInput")
    wq = nc.dram_tensor("wq", [D + 1, DL], dt.bfloat16, kind="ExternalInput")
    wk = nc.dram_tensor("wk", [D + 1, DL], dt.bfloat16, kind="ExternalInput")
    wv = nc.dram_tensor("wv", [D + 1, HL * (DH + 1)], dt.bfloat16,
                        kind="ExternalInput")
    adder = nc.dram_tensor("adder", [P, NT], dt.float32, kind="ExternalInput")
    out = nc.dram_tensor("out", [DL, S], dt.float32, kind="ExternalOutput")

    with tile.TileContext(nc) as tc:
        with (
            tc.tile_pool(name="persist", bufs=1) as sb,
            tc.tile_pool(name="work", bufs=4) as work,
            tc.tile_pool(name="fin", bufs=3) as fin,
            tc.tile_pool(name="ps_s", bufs=2, space="PSUM") as ps_s,
            tc.tile_pool(name="ps_c", bufs=2, space="PSUM") as ps_c,
        ):
            # ---- input DMA (priority order: xT, pair-0 weights first) ----
            xTs = sb.tile([P, KO + 1, S], dt.bfloat16, tag="xTs")
            for ko in range(KO):
                nc.sync.dma_start(
                    xTs[:, ko, :], xT.ap()[ko * P:(ko + 1) * P, :])
            nc.sync.dma_start(xTs[0:1, KO, :], xT.ap()[D:D + 1, :])

            wqs = sb.tile([P, KO + 1, DL], dt.bfloat16, tag="wqs")
            wks = sb.tile([P, KO + 1, DL], dt.bfloat16, tag="wks")
            wvs = sb.tile([P, KO + 1, HL * (DH + 1)], dt.bfloat16, tag="wvs")
            adder_sb = sb.tile([P, NT], dt.float32, tag="adder")

            def dma_w_mtile(w_dram, w_sb, c0, c1):
                nc.sync.dma_start(
                    w_sb[:, 0:KO, c0:c1],
                    w_dram.ap()[0:D, c0:c1].rearrange(
                        "(ko p) m -> p ko m", p=P))
                nc.sync.dma_start(w_sb[0:1, KO, c0:c1],
                                  w_dram.ap()[D:D + 1, c0:c1])

            dma_w_mtile(wq, wqs, 0, P)
            dma_w_mtile(wk, wks, 0, P)
            dma_w_mtile(wv, wvs, 0, 2 * (DH + 1))
            nc.sync.dma_start(adder_sb[:], adder.ap())
            dma_w_mtile(wq, wqs, P, DL)
            dma_w_mtile(wk, wks, P, DL)
            dma_w_mtile(wv, wvs, 2 * (DH + 1), HL * (DH + 1))

            # persistent projection outputs
            qt = sb.tile([P, 3, S], dt.bfloat16, tag="qt")   # Q^T/8 (+bias)
            kt = sb.tile([P, 3, S], dt.bfloat16, tag="kt")   # K^T (+bias)
            vp = sb.tile([P, NT, HL * (DH + 1)], dt.bfloat16, tag="vp")
            # unnormalized ctx' (65 rows per head; row 64 = denominator)
            ctxs = sb.tile([DH + 1, HL, S], dt.float32, tag="ctxs")

            KE = KO + 1 if with_bias else KO  # k-subtiles incl. optional bias

            # ---- projections (psum from ps_c: free at prefix/boundaries,
            # decoupled from the scores/exp slot rotation) ----
            def proj_qk(w_sb, dst, m):
                for n in range(4):
                    pt = ps_c.tile([P, 512], dt.float32, tag="c", name="pt")
                    for k in range(KE):
                        lhsT = (w_sb[:, k, m * P:(m + 1) * P] if k < KO
                                else w_sb[0:1, k, m * P:(m + 1) * P])
                        rhs = (xTs[:, k, n * 512:(n + 1) * 512] if k < KO
                               else xTs[0:1, k, n * 512:(n + 1) * 512])
                        nc.tensor.matmul(pt[:], lhsT, rhs,
                                         start=(k == 0), stop=(k == KE - 1))
                    nc.vector.tensor_copy(dst[:, m, n * 512:(n + 1) * 512],
                                          pt[:])

            def proj_v(mt, hp):
                # always includes the k=768 row: it carries the ones
                # indicator that builds V's 65th (denominator) column.
                w = 2 * (DH + 1)           # one head-pair's V' columns
                c0 = hp * w
                pt = ps_c.tile([P, 512], dt.float32, tag="c", name="pt")
                for k in range(KO + 1):
                    lhsT = (xTs[:, k, mt * P:(mt + 1) * P] if k < KO
                            else xTs[0:1, k, mt * P:(mt + 1) * P])
                    rhs = (wvs[:, k, c0:c0 + w] if k < KO
                           else wvs[0:1, k, c0:c0 + w])
                    nc.tensor.matmul(pt[:, :w], lhsT, rhs,
                                     start=(k == 0), stop=(k == KO))
                nc.vector.tensor_copy(vp[:, mt, c0:c0 + w], pt[:, :w])

            def normalize_seg(hp, fb):
                # normalization of one (pair, F-block) segment; no
                # TensorEngine or PSUM involvement: DMA gather the psum-
                # produced denominators (already staged in ctxs row 64),
                # DVE reciprocal, DMA scatter to a row, gpsimd broadcast,
                # DVE multiply, DMA out.
                nfb = FB // P              # denom cols per head (8)
                den = fin.tile([P, 2 * nfb], dt.float32, tag="den", bufs=2,
                               name="den")
                for hip in range(2):
                    h = 2 * hp + hip
                    nc.sync.dma_start(
                        den[:, hip * nfb:(hip + 1) * nfb],
                        ctxs[DH:DH + 1, h, fb * FB:(fb + 1) * FB])
                rec = fin.tile([P, 2 * nfb], dt.float32, tag="rec", bufs=2,
                               name="rec")
                nc.vector.reciprocal(rec[:], den[:])
                for hip in range(2):
                    h = 2 * hp + hip
                    rr = fin.tile([1, FB], dt.float32, tag="recrow", bufs=2,
                                  name="rr")
                    nc.sync.dma_start(
                        rr[:], rec[:, hip * nfb:(hip + 1) * nfb])
                    rrb = fin.tile([DH, FB], dt.float32, tag="rrb", bufs=2,
                                   name="rrb")
                    nc.gpsimd.partition_broadcast(rrb[:], rr[:])
                    for n in range(NSUB):
                        fcol = fb * FB + n * 512
                        ot = fin.tile([DH, 512], dt.float32, tag="ot",
                                      bufs=3, name="ot")
                        nc.vector.tensor_tensor(
                            ot[:], ctxs[0:DH, h, fcol:fcol + 512],
                            rrb[:, n * 512:(n + 1) * 512], Alu.mult)
                        nc.sync.dma_start(
                            out.ap()[h * DH:(h + 1) * DH, fcol:fcol + 512],
                            ot[:])

            def attn_pair(hp):
                # inner loops for head pair (2hp, 2hp+1); per-ti emission
                # [sA, ctxA(prev), expA, sB, ctxB(prev), expB]: each head's
                # next scores only gate on ITS OWN previous exp (psum slot
                # rotation), so ACT runs back-to-back.
                mtile = hp
                for fb in range(NFB):
                    ctx_ps = [
                        ps_c.tile([DH + 1, FB], dt.float32, tag="c",
                                  name="ctx_ps")
                        for _ in range(2)
                    ]
                    exp_tiles = {}

                    def mm_ctx(hip, ti):
                        h = 2 * hp + hip
                        et = exp_tiles.pop((hip, ti))
                        for n in range(NSUB):
                            nc.tensor.matmul(
                                ctx_ps[hip][:, n * 512:(n + 1) * 512],
                                vp[:, ti, h * (DH + 1):(h + 1) * (DH + 1)],
                                et[:, n * 512:(n + 1) * 512],
                                start=(ti == 0), stop=(ti == NT - 1))

                    for ti in range(NT):
                        for hip in range(2):
                            off = hip * DH
                            s_ps = ps_s.tile([P, FB], dt.float32, tag="s",
                                             name="s_ps")
                            for n in range(NSUB):
                                fcol = fb * FB + n * 512
                                nc.tensor.matmul(
                                    s_ps[:, n * 512:(n + 1) * 512],
                                    kt[off:off + DH, mtile,
                                       ti * P:(ti + 1) * P],
                                    qt[off:off + DH, mtile, fcol:fcol + 512],
                                    start=True, stop=True)
                            if ti > 0:
                                mm_ctx(hip, ti - 1)
                            et = work.tile([P, FB], dt.bfloat16, tag="exp",
                                           name="et")
                            nc.scalar.activation(
                                et[:], s_ps[:], Exp,
                                bias=adder_sb[:, ti:ti + 1], scale=1.0)
                            exp_tiles[(hip, ti)] = et
                    mm_ctx(0, NT - 1)
                    mm_ctx(1, NT - 1)

                    # drain ctx' psum to sbuf staging, then normalize this
                    # segment (all off-PE; overlaps the next segment/pair)
                    for hip in range(2):
                        h = 2 * hp + hip
                        nc.vector.tensor_copy(
                            ctxs[:, h, fb * FB:(fb + 1) * FB],
                            ctx_ps[hip][:])
                    normalize_seg(hp, fb)

            # emission order: minimal prefix (qt/kt m0 + pair-0 V), then
            # pair-0 attention; later projections are boundary blocks that
            # partially hide behind the previous pair's exp backlog.
            proj_qk(wqs, qt, 0)
            proj_qk(wks, kt, 0)
            for mt in range(NT):
                proj_v(mt, 0)
            attn_pair(0)
            proj_qk(wqs, qt, 1)
            proj_qk(wks, kt, 1)
            for mt in range(NT):
                proj_v(mt, 1)
            attn_pair(1)
            proj_qk(wqs, qt, 2)
            proj_qk(wks, kt, 2)
            for mt in range(NT):
                proj_v(mt, 2)
            attn_pair(2)

    nc.compile()
    return nc


def _prep_core_inputs(c, x, Wq, bq, Wk, bk, Wv, bv, mask):
    bf16 = ml_dtypes.bfloat16
    b, hg = c // 2, c % 2
    cols = slice(hg * DL, (hg + 1) * DL)

    xT_aug = np.empty((D + 1, S), dtype=bf16)
    xT_aug[:D] = x[b].T.astype(bf16)
    xT_aug[D] = np.float32(1.0)

    wq_aug = np.empty((D + 1, DL), dtype=bf16)
    wq_aug[:D] = (Wq[:, cols] / 8.0).astype(bf16)
    wq_aug[D] = (bq[cols] / 8.0).astype(bf16)

    wk_aug = np.empty((D + 1, DL), dtype=bf16)
    wk_aug[:D] = Wk[:, cols].astype(bf16)
    wk_aug[D] = bk[cols].astype(bf16)

    wv_aug = np.zeros((D + 1, HL * (DH + 1)), dtype=bf16)
    wv_loc = Wv[:, cols].astype(np.float32)
    bv_loc = bv[cols].astype(np.float32)
    for j in range(HL):
        wv_aug[:D, j * (DH + 1):j * (DH + 1) + DH] = \
            wv_loc[:, j * DH:(j + 1) * DH].astype(bf16)
        wv_aug[D, j * (DH + 1):j * (DH + 1) + DH] = \
            bv_loc[j * DH:(j + 1) * DH].astype(bf16)
        wv_aug[D, j * (DH + 1) + DH] = np.float32(1.0)

    add = ((mask[b].astype(np.float32) - 1.0) * 10000.0)
    adder_t = add.reshape(NT, P).T.copy()   # [128,16]: [p, ti] = add[ti*128+p]

    return {"xT": xT_aug, "wq": wq_aug, "wk": wk_aug, "wv": wv_aug,
            "adder": np.ascontiguousarray(adder_t, dtype=np.float32)}


def kernel(x, Wq, bq, Wk, bk, Wv, bv, mask, _trace=False):
    from concourse.bass_utils import run_bass_kernel_spmd

    x = np.asarray(x, dtype=np.float32)
    Wq = np.asarray(Wq, dtype=np.float32)
    bq = np.asarray(bq, dtype=np.float32)
    Wk = np.asarray(Wk, dtype=np.float32)
    bk = np.asarray(bk, dtype=np.float32)
    Wv = np.asarray(Wv, dtype=np.float32)
    bv = np.asarray(bv, dtype=np.float32)
    mask = np.asarray(mask)

    with_bias = bool(bq.any() or bk.any() or bv.any())
    key = ("nc", with_bias)
    if key not in _CACHE:
        _CACHE[key] = _build(with_bias=with_bias)
    nc = _CACHE[key]

    in_maps = [_prep_core_inputs(c, x, Wq, bq, Wk, bk, Wv, bv, mask)
               for c in range(NCORES)]
    res = run_bass_kernel_spmd(nc, in_maps, core_ids=list(range(NCORES)),
                               trace=_trace)
    if _trace:
        _CACHE["last_result"] = res

    full = np.empty((B, S, D), dtype=np.float32)
    for c in range(NCORES):
        b, hg = c // 2, c % 2
        full[b, :, hg * DL:(hg + 1) * DL] = res.results[c]["out"].T
    return full
